# revision 4
# baseline (speedup 1.0000x reference)
"""Multi-head causal attention (dense transformer block) on 8 TRN2 NeuronCores.

Sharding: core c -> (batch b = c//2, head-group g = c%2).  Each core computes
the QKV projection for its 8 heads (column-parallel), full causal attention for
those heads, and the out-projection partial over its 1024 channels
(row-parallel).  A pairwise AllReduce over cores (2b, 2b+1) completes the
out-projection; the host reads the reduced result from the even core of each
pair.

On-chip layout notes:
 - q/k are produced TRANSPOSED ([head_dim, seq]) so attention scores come out
   as S^T = K @ Q^T with softmax along the PSUM partition dim; the softmax
   denominator is a ones-vector matmul, so no on-chip transposes are needed.
 - exp() has no max-subtraction: scores are O(+-20) for this data
   distribution, safely inside fp32/bf16 exp range.
 - all matmuls run in bf16 with fp32 PSUM accumulation; softmax
   normalization runs in fp32.
All host-side tensor reshapes below exist to make every DMA read contiguous
per SBUF partition line.
"""

import math
import sys
import types
from contextlib import ExitStack

sys.path.insert(0, "/opt/trn_rl_repo")

import ml_dtypes
import numpy as np

import concourse.bass as bass
import concourse.mybir as mybir
import concourse.tile as tile
from concourse import bass_utils
from concourse.vector_clock import ScopedClock

BF16 = mybir.dt.bfloat16
F32 = mybir.dt.float32
NPBF16 = ml_dtypes.bfloat16

HD = 128  # head dim
SQB = 512  # seq block (matmul moving free dim)
INV_SQRT_HD = 1.0 / math.sqrt(HD)


MAX_WAITS = 1  # walrus here rejects multi-wait instructions


def _split_excess_waits(nc):
    """Walrus here encodes at most MAX_WAITS sem-waits per instruction.  Move
    any excess onto same-engine NoOps inserted immediately before the
    instruction — the engine still observes every wait before executing it."""
    import bass_rust

    for f in nc.m.functions:
        for bb in f.blocks:
            out = []
            changed = False
            for inst in bb.instructions:
                si = inst.sync_info
                waits = list(si.on_wait) if si is not None else []
                if len(waits) > MAX_WAITS:
                    changed = True
                    excess, keep = waits[:-MAX_WAITS], waits[-MAX_WAITS:]
                    for i in range(0, len(excess), MAX_WAITS):
                        nop = mybir.InstNoOp(
                            name=f"waitnop-{nc.next_id()}", ins=[], outs=[]
                        )
                        nop.engine = inst.engine
                        nop.sync_info = bass_rust.SyncInfo(
                            on_wait=excess[i : i + MAX_WAITS], on_update=[]
                        )
                        nc.register_instruction(nop)
                        out.append(nop)
                    inst.sync_info.on_wait = keep
                out.append(inst)
            if changed:
                bb.instructions = out


class TileContextFixed(tile.TileContext):
    def _drain_and_barrier(self, tick_clock, wait_clock):
        super()._drain_and_barrier(tick_clock, wait_clock)
        _split_excess_waits(self.nc)


def build_program(S, D, HL, n_cores):
    """Emit the SPMD per-core program.  S: seq len, D: model dim, HL: heads
    per core.  Every core runs the identical graph on different data."""
    DT = D // 128  # contraction tiles over model dim
    SB = S // SQB  # seq blocks
    ST = S // 128  # seq tiles
    CH = HL * HD  # local out-projection channels
    CT = CH // 128  # channel tiles
    RT = 2 * HL  # q/k row tiles ([q_h, k_h] per head)
    OB = D // SQB  # out-projection column blocks
    VB = CH // SQB  # v column blocks
    assert VB >= 1 and SB >= 1 and OB >= 1

    nc = bass.Bass(num_devices=n_cores)

    # ---- per-core external tensors (all host-pretiled, bf16) ----
    xt1 = nc.dram_tensor("xt1", [SB, 128, DT, SQB], BF16, kind="ExternalInput")
    xt2 = nc.dram_tensor("xt2", [ST, 128, DT, 128], BF16, kind="ExternalInput")
    wqk = nc.dram_tensor("wqk", [RT, 128, DT, 128], BF16, kind="ExternalInput")
    wv = nc.dram_tensor("wv", [128, DT, CH], BF16, kind="ExternalInput")
    wo = nc.dram_tensor("wo", [OB, 128, CT, SQB], BF16, kind="ExternalInput")
    y_ext = nc.dram_tensor("y", [S, D], F32, kind="ExternalOutput")

    with TileContextFixed(nc) as tc, ExitStack() as top:
        dram = top.enter_context(tc.tile_pool(name="dram", bufs=1, space="DRAM"))
        y_stage = dram.tile([S, D], F32, name="y_stage")
        y_red = dram.tile([S, D], F32, name="y_red")

        const_pool = top.enter_context(tc.tile_pool(name="const", bufs=1))
        qk_pool = top.enter_context(tc.tile_pool(name="qkT", bufs=1))
        v_pool = top.enter_context(tc.tile_pool(name="vres", bufs=1))

        ps_pool = top.enter_context(tc.tile_pool(name="ps", bufs=2, space="PSUM"))
        acc_pool = top.enter_context(tc.tile_pool(name="acc", bufs=2, space="PSUM"))
        dn_pool = top.enter_context(tc.tile_pool(name="dn", bufs=2, space="PSUM"))
        bc_pool = top.enter_context(tc.tile_pool(name="bc", bufs=2, space="PSUM"))

        # ---- constants ----
        ones_col = const_pool.tile([128, 1], BF16, name="ones_col")
        nc.gpsimd.memset(ones_col[:], 1.0)
        ones_row = const_pool.tile([1, 128], F32, name="ones_row")
        nc.gpsimd.memset(ones_row[:], 1.0)
        # 4 diagonal causal masks (multiplicative, applied post-exp):
        # mask_j[k, q] = 1 if q - k - j*128 >= 0 else 0, on [128, SQB] tiles.
        masks = []
        for j in range(SQB // 128):
            mf = const_pool.tile([128, SQB], F32, name=f"maskf{j}")
            nc.gpsimd.memset(mf[:], 1.0)
            nc.gpsimd.affine_select(
                out=mf[:],
                in_=mf[:],
                pattern=[[1, SQB]],
                compare_op=mybir.AluOpType.is_ge,
                fill=0.0,
                base=-j * 128,
                channel_multiplier=-1,
            )
            mb = const_pool.tile([128, SQB], BF16, name=f"mask{j}")
            nc.vector.tensor_copy(mb[:], mf[:])
            masks.append(mb)

        # ---- persistent intermediates ----
        qkT = [
            qk_pool.tile([128, S], BF16, name=f"qkT{rt}", tag=f"qkT{rt}")
            for rt in range(RT)
        ]
        vres = [
            v_pool.tile([128, CH], BF16, name=f"v{st}", tag=f"v{st}")
            for st in range(ST)
        ]

        # ================= phase 1: projections =================
        with ExitStack() as ph1:
            xsb_pool = ph1.enter_context(tc.tile_pool(name="xsb", bufs=2))
            wqk_pool = ph1.enter_context(tc.tile_pool(name="wqkp", bufs=2))
            wv_pool = ph1.enter_context(tc.tile_pool(name="wvp", bufs=1))
            xst_pool = ph1.enter_context(tc.tile_pool(name="xst", bufs=2))

            # --- 1a: q/k rows, transposed layout qkT[row, s] ---
            for sb in range(SB):
                xsb = xsb_pool.tile([128, DT, SQB], BF16, name=f"xsb{sb}", tag="xsb")
                nc.sync.dma_start(xsb[:], xt1[sb])
                for rt in range(RT):
                    wq = wqk_pool.tile(
                        [128, DT, 128], BF16, name=f"wq{sb}_{rt}", tag="wq"
                    )
                    nc.sync.dma_start(wq[:], wqk[rt])
                    ps = ps_pool.tile([128, SQB], F32, name=f"psqk{sb}_{rt}", tag="ps")
                    for d in range(DT):
                        nc.tensor.matmul(
                            ps[:],
                            lhsT=wq[:, d, :],
                            rhs=xsb[:, d, :],
                            start=(d == 0),
                            stop=(d == DT - 1),
                        )
                    nc.scalar.copy(qkT[rt][:, sb * SQB : (sb + 1) * SQB], ps[:])

            # --- 1b: v rows, natural layout v[s, ch] ---
            wvt = wv_pool.tile([128, DT, CH], BF16, name="wvt")
            nc.sync.dma_start(wvt[:], wv[:])
            for st in range(ST):
                xst = xst_pool.tile([128, DT, 128], BF16, name=f"xst{st}", tag="xst")
                nc.sync.dma_start(xst[:], xt2[st])
                for vb in range(VB):
                    ps = ps_pool.tile([128, SQB], F32, name=f"psv{st}_{vb}", tag="ps")
                    for d in range(DT):
                        nc.tensor.matmul(
                            ps[:],
                            lhsT=xst[:, d, :],
                            rhs=wvt[:, d, vb * SQB : (vb + 1) * SQB],
                            start=(d == 0),
                            stop=(d == DT - 1),
                        )
                    nc.scalar.copy(vres[st][:, vb * SQB : (vb + 1) * SQB], ps[:])

        # ================= phase 2: attention =================
        with ExitStack() as ph2:
            e_pool = ph2.enter_context(tc.tile_pool(name="e", bufs=4))
            ao_pool = ph2.enter_context(tc.tile_pool(name="ao", bufs=1))
            r_pool = ph2.enter_context(tc.tile_pool(name="r", bufs=2))

            aoT = [
                ao_pool.tile([128, S], BF16, name=f"aoT{h}", tag=f"aoT{h}")
                for h in range(HL)
            ]

            def emit_normalize(blk):
                """softmax denominator division for a finished (h, sqb) block"""
                h, sqb, ot, dn = blk
                r = r_pool.tile([1, SQB], F32, name=f"r{h}_{sqb}", tag="r")
                nc.vector.reciprocal(r[:], dn[:])
                rb = bc_pool.tile([128, SQB], F32, name=f"rb{h}_{sqb}", tag="rb")
                nc.tensor.matmul(rb[:], lhsT=ones_row[:], rhs=r[:], start=True, stop=True)
                rs = r_pool.tile([128, SQB], F32, name=f"rs{h}_{sqb}", tag="rs")
                nc.scalar.copy(rs[:], rb[:])
                nc.vector.tensor_mul(
                    aoT[h][:, sqb * SQB : (sqb + 1) * SQB], ot[:], rs[:]
                )

            prev_blk = None
            for h in range(HL):
                qT = qkT[2 * h]
                kT = qkT[2 * h + 1]
                for sqb in range(SB):
                    n_sk = (sqb + 1) * (SQB // 128)
                    diag0 = sqb * (SQB // 128)
                    ot = acc_pool.tile(
                        [128, SQB], F32, name=f"ot{h}_{sqb}", tag="ot"
                    )
                    dn = dn_pool.tile([1, SQB], F32, name=f"dn{h}_{sqb}", tag="dn")
                    pend = []  # exp tiles awaiting denominator/PV matmuls

                    def flush_one():
                        skt, et = pend.pop(0)
                        nc.tensor.matmul(
                            dn[:],
                            lhsT=ones_col[:],
                            rhs=et[:],
                            start=(skt == 0),
                            stop=(skt == n_sk - 1),
                        )
                        nc.tensor.matmul(
                            ot[:],
                            lhsT=vres[skt][:, h * HD : (h + 1) * HD],
                            rhs=et[:],
                            start=(skt == 0),
                            stop=(skt == n_sk - 1),
                        )

                    for skt in range(n_sk):
                        ps = ps_pool.tile(
                            [128, SQB], F32, name=f"s{h}_{sqb}_{skt}", tag="ps"
                        )
                        nc.tensor.matmul(
                            ps[:],
                            lhsT=kT[:, skt * 128 : (skt + 1) * 128],
                            rhs=qT[:, sqb * SQB : (sqb + 1) * SQB],
                            start=True,
                            stop=True,
                        )
                        e = e_pool.tile(
                            [128, SQB], BF16, name=f"e{h}_{sqb}_{skt}", tag="e"
                        )
                        nc.scalar.activation(
                            e[:],
                            ps[:],
                            mybir.ActivationFunctionType.Exp,
                            scale=INV_SQRT_HD,
                        )
                        if skt >= diag0:
                            em = e_pool.tile(
                                [128, SQB], BF16, name=f"em{h}_{sqb}_{skt}", tag="em"
                            )
                            nc.vector.tensor_mul(em[:], e[:], masks[skt - diag0][:])
                            e = em
                        pend.append((skt, e))
                        # one-stage delay keeps PE from stalling on ScalarE exp
                        if len(pend) > 1:
                            flush_one()
                        if skt == 0 and prev_blk is not None:
                            emit_normalize(prev_blk)
                            prev_blk = None
                    while pend:
                        flush_one()
                    prev_blk = (h, sqb, ot, dn)
            emit_normalize(prev_blk)

            # ================= phase 3: out-projection partial =================
            with ExitStack() as ph3:
                wo_pool = ph3.enter_context(tc.tile_pool(name="wop", bufs=2))
                y_pool = ph3.enter_context(tc.tile_pool(name="ysb", bufs=3))
                for ob in range(OB):
                    wot = wo_pool.tile([128, CT, SQB], BF16, name=f"wo{ob}", tag="wo")
                    nc.sync.dma_start(wot[:], wo[ob])
                    for st in range(ST):
                        ps = ps_pool.tile(
                            [128, SQB], F32, name=f"py{ob}_{st}", tag="ps"
                        )
                        for ct in range(CT):
                            nc.tensor.matmul(
                                ps[:],
                                lhsT=aoT[ct][:, st * 128 : (st + 1) * 128],
                                rhs=wot[:, ct, :],
                                start=(ct == 0),
                                stop=(ct == CT - 1),
                            )
                        ysb = y_pool.tile([128, SQB], F32, name=f"y{ob}_{st}", tag="y")
                        nc.scalar.copy(ysb[:], ps[:])
                        nc.sync.dma_start(
                            y_stage[
                                st * 128 : (st + 1) * 128, ob * SQB : (ob + 1) * SQB
                            ],
                            ysb[:],
                        )

        # ================= phase 4: pairwise reduce + output =================
        groups = [[2 * i, 2 * i + 1] for i in range(n_cores // 2)]
        nc.gpsimd.collective_compute(
            "AllReduce",
            mybir.AluOpType.add,
            replica_groups=groups,
            ins=[y_stage.opt()],
            outs=[y_red.opt()],
        )
        nc.sync.dma_start(y_ext[:], y_red[:])

    return nc


# ------------------------- host-side data prep -------------------------


def _pretile_x(xb, DT, SB, ST):
    """x[b] [S, D] f32 -> (xt1 [SB,128,DT,SQB], xt2 [ST,128,DT,128]) bf16"""
    xT = np.ascontiguousarray(xb.T).astype(NPBF16)  # [D, S]
    xt1 = np.ascontiguousarray(
        xT.reshape(DT, 128, SB, SQB).transpose(2, 1, 0, 3)
    )
    xt2 = np.ascontiguousarray(
        xT.reshape(DT, 128, ST, 128).transpose(2, 1, 0, 3)
    )
    return xt1, xt2


def _pretile_weights(w_project, w_out, D, HL, g):
    """Per-core weight tilings for head-group g (HL heads)."""
    DT = D // 128
    CH = HL * HD
    CT = CH // 128
    RT = 2 * HL
    OB = D // SQB
    h0 = g * HL
    # q/k rows interleaved per head: [q_h, k_h] blocks of 128 rows
    rows = []
    for h in range(h0, h0 + HL):
        rows.append(w_project[h * HD : (h + 1) * HD])
        rows.append(w_project[D + h * HD : D + (h + 1) * HD])
    wqk_rows = np.concatenate(rows, axis=0)  # [2*CH, D]
    wqk = np.ascontiguousarray(
        wqk_rows.reshape(RT, 128, DT, 128).transpose(0, 3, 2, 1)
    ).astype(NPBF16)
    wv_rows = w_project[2 * D + h0 * HD : 2 * D + (h0 + HL) * HD]  # [CH, D]
    # -> [p, t, vr]: WvT[d, vr] = wv_rows[vr, d]; build [128, DT, CH]
    wv = np.ascontiguousarray(
        wv_rows.reshape(CT, 128, DT, 128).transpose(3, 2, 0, 1).reshape(128, DT, CH)
    ).astype(NPBF16)
    woT = w_out[:, h0 * HD : h0 * HD + CH].T  # [CH, D]
    wo = np.ascontiguousarray(
        woT.reshape(CT, 128, OB, SQB).transpose(2, 1, 0, 3)
    ).astype(NPBF16)
    return wqk, wv, wo


_BUILD_CACHE = {}


def _get_program(S, D, HL, n_cores):
    key = (S, D, HL, n_cores)
    if key not in _BUILD_CACHE:
        _BUILD_CACHE[key] = build_program(S, D, HL, n_cores)
    return _BUILD_CACHE[key]


def _install_ntff_hook():
    """Best-effort: register the axon NTFF profiling hook so callers can pass
    trace=True to run_bass_kernel_spmd.  No-op if unavailable."""
    try:
        import antenv

        if "antenv.axon_hooks" not in sys.modules:
            mod = types.ModuleType("antenv.axon_hooks")
            holder = [None]
            mod.set_axon_ntff_profile_hook = lambda h: holder.__setitem__(0, h)
            mod.get_axon_ntff_profile_hook = lambda: holder[0]
            sys.modules["antenv.axon_hooks"] = mod
            antenv.axon_hooks = mod
            from trn_agent_boot.trn_boot import _ntff_profile_via_ctypes

            hook = _ntff_profile_via_ctypes("/opt/axon/libaxon_pjrt.so")
            mod.set_axon_ntff_profile_hook(hook)
    except Exception:
        pass


def run(x, w_project, w_out, trace=False):
    """Run the sharded kernel on hardware; returns (y [B,S,D] f32, results)."""
    x = np.asarray(x, dtype=np.float32)
    w_project = np.asarray(w_project, dtype=np.float32)
    w_out = np.asarray(w_out, dtype=np.float32)
    B, S, D = x.shape
    H = w_project.shape[0] // 3 // HD  # total heads
    HL = H // 2  # heads per core (2 cores per batch)
    n_cores = 2 * B
    DT, SB, ST = D // 128, S // SQB, S // 128

    nc = _get_program(S, D, HL, n_cores)

    in_maps = []
    for b in range(B):
        xt1, xt2 = _pretile_x(x[b], DT, SB, ST)
        for g in range(2):
            wqk, wv, wo = _pretile_weights(w_project, w_out, D, HL, g)
            in_maps.append({"xt1": xt1, "xt2": xt2, "wqk": wqk, "wv": wv, "wo": wo})

    if trace:
        _install_ntff_hook()
    res = bass_utils.run_bass_kernel_spmd(
        nc, in_maps, core_ids=list(range(n_cores)), trace=trace
    )
    y = np.stack([res.results[2 * b]["y"] for b in range(B)]).astype(np.float32)
    return y, res


def kernel(x, w_project, w_out):
    y, _ = run(x, w_project, w_out, trace=False)
    return y


# revision 10
# speedup vs baseline: 1.4493x; 1.4493x over previous
"""Multi-head causal attention (dense transformer block) on 8 TRN2 NeuronCores.

Sharding: core c -> (batch b = c//2, head-group g = c%2).  Each core computes
the QKV projection for its 8 heads (column-parallel), full causal attention for
those heads, and the out-projection partial over its 1024 channels
(row-parallel).  A pairwise AllReduce over cores (2b, 2b+1) completes the
out-projection; the host reads the reduced result from the even core of each
pair.

On-chip layout notes:
 - q/k are produced TRANSPOSED ([head_dim, seq]) so attention scores come out
   as S^T = K @ Q^T with softmax along the PSUM partition dim; the softmax
   denominator is a ones-vector matmul, so no on-chip transposes are needed.
 - exp() has no max-subtraction: scores are O(+-20) for this data
   distribution, safely inside fp32/bf16 exp range.
 - all matmuls run in bf16 with fp32 PSUM accumulation; softmax
   normalization runs in fp32.
All host-side tensor reshapes below exist to make every DMA read contiguous
per SBUF partition line.
"""

import math
import sys
import types
from contextlib import ExitStack

sys.path.insert(0, "/opt/trn_rl_repo")

import ml_dtypes
import numpy as np

import concourse.bass as bass
import concourse.mybir as mybir
import concourse.tile as tile
from concourse import bass_utils
from concourse.vector_clock import ScopedClock

BF16 = mybir.dt.bfloat16
F32 = mybir.dt.float32
NPBF16 = ml_dtypes.bfloat16

HD = 128  # head dim
SQB = 512  # seq block (matmul moving free dim)
INV_SQRT_HD = 1.0 / math.sqrt(HD)


MAX_WAITS = 1  # walrus here rejects multi-wait instructions


def _split_excess_waits(nc):
    """Walrus here encodes at most MAX_WAITS sem-waits per instruction.  Move
    any excess onto same-engine NoOps inserted immediately before the
    instruction — the engine still observes every wait before executing it."""
    import bass_rust

    for f in nc.m.functions:
        for bb in f.blocks:
            out = []
            changed = False
            for inst in bb.instructions:
                si = inst.sync_info
                waits = list(si.on_wait) if si is not None else []
                if len(waits) > MAX_WAITS:
                    changed = True
                    excess, keep = waits[:-MAX_WAITS], waits[-MAX_WAITS:]
                    for i in range(0, len(excess), MAX_WAITS):
                        nop = mybir.InstNoOp(
                            name=f"waitnop-{nc.next_id()}", ins=[], outs=[]
                        )
                        nop.engine = inst.engine
                        nop.sync_info = bass_rust.SyncInfo(
                            on_wait=excess[i : i + MAX_WAITS], on_update=[]
                        )
                        nc.register_instruction(nop)
                        out.append(nop)
                    inst.sync_info.on_wait = keep
                out.append(inst)
            if changed:
                bb.instructions = out


class TileContextFixed(tile.TileContext):
    def _drain_and_barrier(self, tick_clock, wait_clock):
        super()._drain_and_barrier(tick_clock, wait_clock)
        _split_excess_waits(self.nc)


def build_program(S, D, HL, n_cores):
    """Emit the SPMD per-core program.  S: seq len, D: model dim, HL: heads
    per core.  Every core runs the identical graph on different data."""
    DT = D // 128  # contraction tiles over model dim
    SB = S // SQB  # seq blocks
    ST = S // 128  # seq tiles
    CH = HL * HD  # local out-projection channels
    CT = CH // 128  # channel tiles
    RT = 2 * HL  # q/k row tiles ([q_h, k_h] per head)
    OB = D // SQB  # out-projection column blocks
    VB = CH // SQB  # v column blocks
    assert VB >= 1 and SB >= 1 and OB >= 1

    STG = 4  # seq tiles per ReduceScatter chunk
    G = ST // STG  # collective chunks
    assert ST % STG == 0

    nc = bass.Bass(num_devices=n_cores)

    # ---- per-core external tensors (all host-pretiled, bf16) ----
    xt1 = nc.dram_tensor("xt1", [SB, 128, DT, SQB], BF16, kind="ExternalInput")
    xt2 = nc.dram_tensor("xt2", [ST, 128, DT, 128], BF16, kind="ExternalInput")
    wqk = nc.dram_tensor("wqk", [RT, 128, DT, 128], BF16, kind="ExternalInput")
    wv = nc.dram_tensor("wv", [128, DT, CH], BF16, kind="ExternalInput")
    wo = nc.dram_tensor("wo", [OB, 128, CT, SQB], BF16, kind="ExternalInput")
    y_ext = nc.dram_tensor("y", [S // 2, D], F32, kind="ExternalOutput")

    with TileContextFixed(nc) as tc, ExitStack() as top:
        dram = top.enter_context(tc.tile_pool(name="dram", bufs=1, space="DRAM"))
        CR = STG * 128  # rows per collective chunk
        y_stage = [
            dram.tile([CR, D], F32, name=f"y_stage{g}", tag=f"ystage{g}")
            for g in range(G)
        ]
        y_red = [
            dram.tile([CR // 2, D], F32, name=f"y_red{g}", tag=f"yred{g}")
            for g in range(G)
        ]

        const_pool = top.enter_context(tc.tile_pool(name="const", bufs=1))
        qk_pool = top.enter_context(tc.tile_pool(name="qkT", bufs=1))
        v_pool = top.enter_context(tc.tile_pool(name="vres", bufs=1))

        ps_pool = top.enter_context(tc.tile_pool(name="ps", bufs=4, space="PSUM"))
        acc_pool = top.enter_context(tc.tile_pool(name="acc", bufs=2, space="PSUM"))
        dn_pool = top.enter_context(tc.tile_pool(name="dn", bufs=2, space="PSUM"))

        # ---- constants ----
        # all-ones stationary: the denominator matmul ones128.T @ E produces
        # the softmax denominator replicated across all 128 PSUM partitions
        # (same cycle cost as a [*,1] output) — broadcast comes for free.
        ones128 = const_pool.tile([128, 128], BF16, name="ones128")
        nc.gpsimd.memset(ones128[:], 1.0)
        # 4 diagonal causal masks (multiplicative, applied post-exp):
        # mask_j[k, q] = 1 if q - k - j*128 >= 0 else 0, on [128, SQB] tiles.
        masks = []
        for j in range(SQB // 128):
            mf = const_pool.tile([128, SQB], F32, name=f"maskf{j}")
            nc.gpsimd.memset(mf[:], 1.0)
            nc.gpsimd.affine_select(
                out=mf[:],
                in_=mf[:],
                pattern=[[1, SQB]],
                compare_op=mybir.AluOpType.is_ge,
                fill=0.0,
                base=-j * 128,
                channel_multiplier=-1,
            )
            mb = const_pool.tile([128, SQB], BF16, name=f"mask{j}")
            nc.vector.tensor_copy(mb[:], mf[:])
            masks.append(mb)

        # ---- persistent intermediates ----
        qkT = [
            qk_pool.tile([128, S], BF16, name=f"qkT{rt}", tag=f"qkT{rt}")
            for rt in range(RT)
        ]
        vres = [
            v_pool.tile([128, CH], BF16, name=f"v{st}", tag=f"v{st}")
            for st in range(ST)
        ]

        # ================= phase 1: projections =================
        with ExitStack() as ph1:
            xsb_pool = ph1.enter_context(tc.tile_pool(name="xsb", bufs=2))
            wqk_pool = ph1.enter_context(tc.tile_pool(name="wqkp", bufs=2))
            wv_pool = ph1.enter_context(tc.tile_pool(name="wvp", bufs=1))
            xst_pool = ph1.enter_context(tc.tile_pool(name="xst", bufs=2))

            # --- 1a: q/k rows, transposed layout qkT[row, s] ---
            for sb in range(SB):
                xsb = xsb_pool.tile([128, DT, SQB], BF16, name=f"xsb{sb}", tag="xsb")
                nc.sync.dma_start(xsb[:], xt1[sb])
                for rt in range(RT):
                    wq = wqk_pool.tile(
                        [128, DT, 128], BF16, name=f"wq{sb}_{rt}", tag="wq"
                    )
                    nc.sync.dma_start(wq[:], wqk[rt])
                    ps = ps_pool.tile([128, SQB], F32, name=f"psqk{sb}_{rt}", tag="ps")
                    for d in range(DT):
                        nc.tensor.matmul(
                            ps[:],
                            lhsT=wq[:, d, :],
                            rhs=xsb[:, d, :],
                            start=(d == 0),
                            stop=(d == DT - 1),
                        )
                    nc.scalar.copy(qkT[rt][:, sb * SQB : (sb + 1) * SQB], ps[:])

            # --- 1b: v rows, natural layout v[s, ch] ---
            wvt = wv_pool.tile([128, DT, CH], BF16, name="wvt")
            nc.sync.dma_start(wvt[:], wv[:])
            for st in range(ST):
                xst = xst_pool.tile([128, DT, 128], BF16, name=f"xst{st}", tag="xst")
                nc.sync.dma_start(xst[:], xt2[st])
                for vb in range(VB):
                    ps = ps_pool.tile([128, SQB], F32, name=f"psv{st}_{vb}", tag="ps")
                    for d in range(DT):
                        nc.tensor.matmul(
                            ps[:],
                            lhsT=xst[:, d, :],
                            rhs=wvt[:, d, vb * SQB : (vb + 1) * SQB],
                            start=(d == 0),
                            stop=(d == DT - 1),
                        )
                    nc.scalar.copy(vres[st][:, vb * SQB : (vb + 1) * SQB], ps[:])

        # ================= phase 2: attention =================
        with ExitStack() as ph2:
            e_pool = ph2.enter_context(tc.tile_pool(name="e", bufs=4))
            ao_pool = ph2.enter_context(tc.tile_pool(name="ao", bufs=1))
            r_pool = ph2.enter_context(tc.tile_pool(name="r", bufs=2))

            aoT = [
                ao_pool.tile([128, S], BF16, name=f"aoT{h}", tag=f"aoT{h}")
                for h in range(HL)
            ]

            def emit_normalize(blk):
                """softmax denominator division for a finished (h, sqb) block.
                dn is already partition-broadcast, so this is DVE-only — the
                TensorEngine never waits on it."""
                h, sqb, ot, dn = blk
                r = r_pool.tile([128, SQB], F32, name=f"r{h}_{sqb}", tag="r")
                nc.vector.reciprocal(r[:], dn[:])
                nc.vector.tensor_mul(
                    aoT[h][:, sqb * SQB : (sqb + 1) * SQB], ot[:], r[:]
                )

            prev_blk = None
            for h in range(HL):
                qT = qkT[2 * h]
                kT = qkT[2 * h + 1]
                for sqb in range(SB):
                    n_sk = (sqb + 1) * (SQB // 128)
                    diag0 = sqb * (SQB // 128)
                    ot = acc_pool.tile(
                        [128, SQB], F32, name=f"ot{h}_{sqb}", tag="ot"
                    )
                    dn = dn_pool.tile([128, SQB], F32, name=f"dn{h}_{sqb}", tag="dn")
                    pend = []  # exp tiles awaiting denominator/PV matmuls

                    def flush_one():
                        skt, et = pend.pop(0)
                        nc.tensor.matmul(
                            dn[:],
                            lhsT=ones128[:],
                            rhs=et[:],
                            start=(skt == 0),
                            stop=(skt == n_sk - 1),
                        )
                        nc.tensor.matmul(
                            ot[:],
                            lhsT=vres[skt][:, h * HD : (h + 1) * HD],
                            rhs=et[:],
                            start=(skt == 0),
                            stop=(skt == n_sk - 1),
                        )

                    for skt in range(n_sk):
                        ps = ps_pool.tile(
                            [128, SQB], F32, name=f"s{h}_{sqb}_{skt}", tag="ps"
                        )
                        nc.tensor.matmul(
                            ps[:],
                            lhsT=kT[:, skt * 128 : (skt + 1) * 128],
                            rhs=qT[:, sqb * SQB : (sqb + 1) * SQB],
                            start=True,
                            stop=True,
                        )
                        e = e_pool.tile(
                            [128, SQB], BF16, name=f"e{h}_{sqb}_{skt}", tag="e"
                        )
                        nc.scalar.activation(
                            e[:],
                            ps[:],
                            mybir.ActivationFunctionType.Exp,
                            scale=INV_SQRT_HD,
                        )
                        if skt >= diag0:
                            em = e_pool.tile(
                                [128, SQB], BF16, name=f"em{h}_{sqb}_{skt}", tag="em"
                            )
                            nc.vector.tensor_mul(em[:], e[:], masks[skt - diag0][:])
                            e = em
                        pend.append((skt, e))
                        # one-stage delay keeps PE from stalling on ScalarE exp
                        if len(pend) > 1:
                            flush_one()
                    while pend:
                        flush_one()
                    emit_normalize((h, sqb, ot, dn))

            # ========= phase 3: out-projection partial + chunked reduce =========
            # Row-group-major loop so each group's pairwise ReduceScatter fires
            # while later groups are still computing (collective overlaps PE).
            groups = [[2 * i, 2 * i + 1] for i in range(n_cores // 2)]
            with ExitStack() as ph3:
                wo_pool = ph3.enter_context(tc.tile_pool(name="wop", bufs=1))
                y_pool = ph3.enter_context(tc.tile_pool(name="ysb", bufs=3))
                wots = []
                for ob in range(OB):
                    wot = wo_pool.tile(
                        [128, CT, SQB], BF16, name=f"wo{ob}", tag=f"wo{ob}"
                    )
                    nc.sync.dma_start(wot[:], wo[ob])
                    wots.append(wot)
                for g in range(G):
                    for sti in range(STG):
                        st = g * STG + sti
                        for ob in range(OB):
                            ps = ps_pool.tile(
                                [128, SQB], F32, name=f"py{ob}_{st}", tag="ps"
                            )
                            for ct in range(CT):
                                nc.tensor.matmul(
                                    ps[:],
                                    lhsT=aoT[ct][:, st * 128 : (st + 1) * 128],
                                    rhs=wots[ob][:, ct, :],
                                    start=(ct == 0),
                                    stop=(ct == CT - 1),
                                )
                            ysb = y_pool.tile(
                                [128, SQB], F32, name=f"y{ob}_{st}", tag="y"
                            )
                            nc.scalar.copy(ysb[:], ps[:])
                            nc.sync.dma_start(
                                y_stage[g][
                                    sti * 128 : (sti + 1) * 128,
                                    ob * SQB : (ob + 1) * SQB,
                                ],
                                ysb[:],
                            )
                    # pairwise ReduceScatter of this row group: rank 0 (even
                    # core) receives the first CR//2 reduced rows, rank 1 the
                    # rest; each core outputs its half.
                    nc.gpsimd.collective_compute(
                        "ReduceScatter",
                        mybir.AluOpType.add,
                        replica_groups=groups,
                        ins=[y_stage[g].opt()],
                        outs=[y_red[g].opt()],
                    )
                    nc.sync.dma_start(
                        y_ext[g * (CR // 2) : (g + 1) * (CR // 2), :], y_red[g][:]
                    )

    return nc


# ------------------------- host-side data prep -------------------------


def _pretile_x(xb, DT, SB, ST):
    """x[b] [S, D] f32 -> (xt1 [SB,128,DT,SQB], xt2 [ST,128,DT,128]) bf16"""
    xT = np.ascontiguousarray(xb.T).astype(NPBF16)  # [D, S]
    xt1 = np.ascontiguousarray(
        xT.reshape(DT, 128, SB, SQB).transpose(2, 1, 0, 3)
    )
    xt2 = np.ascontiguousarray(
        xT.reshape(DT, 128, ST, 128).transpose(2, 1, 0, 3)
    )
    return xt1, xt2


def _pretile_weights(w_project, w_out, D, HL, g):
    """Per-core weight tilings for head-group g (HL heads)."""
    DT = D // 128
    CH = HL * HD
    CT = CH // 128
    RT = 2 * HL
    OB = D // SQB
    h0 = g * HL
    # q/k rows interleaved per head: [q_h, k_h] blocks of 128 rows
    rows = []
    for h in range(h0, h0 + HL):
        rows.append(w_project[h * HD : (h + 1) * HD])
        rows.append(w_project[D + h * HD : D + (h + 1) * HD])
    wqk_rows = np.concatenate(rows, axis=0)  # [2*CH, D]
    wqk = np.ascontiguousarray(
        wqk_rows.reshape(RT, 128, DT, 128).transpose(0, 3, 2, 1)
    ).astype(NPBF16)
    wv_rows = w_project[2 * D + h0 * HD : 2 * D + (h0 + HL) * HD]  # [CH, D]
    # -> [p, t, vr]: WvT[d, vr] = wv_rows[vr, d]; build [128, DT, CH]
    wv = np.ascontiguousarray(
        wv_rows.reshape(CT, 128, DT, 128).transpose(3, 2, 0, 1).reshape(128, DT, CH)
    ).astype(NPBF16)
    woT = w_out[:, h0 * HD : h0 * HD + CH].T  # [CH, D]
    wo = np.ascontiguousarray(
        woT.reshape(CT, 128, OB, SQB).transpose(2, 1, 0, 3)
    ).astype(NPBF16)
    return wqk, wv, wo


_BUILD_CACHE = {}


def _get_program(S, D, HL, n_cores):
    key = (S, D, HL, n_cores)
    if key not in _BUILD_CACHE:
        _BUILD_CACHE[key] = build_program(S, D, HL, n_cores)
    return _BUILD_CACHE[key]


def _install_ntff_hook():
    """Best-effort: register the axon NTFF profiling hook so callers can pass
    trace=True to run_bass_kernel_spmd.  No-op if unavailable."""
    try:
        import antenv

        if "antenv.axon_hooks" not in sys.modules:
            mod = types.ModuleType("antenv.axon_hooks")
            holder = [None]
            mod.set_axon_ntff_profile_hook = lambda h: holder.__setitem__(0, h)
            mod.get_axon_ntff_profile_hook = lambda: holder[0]
            sys.modules["antenv.axon_hooks"] = mod
            antenv.axon_hooks = mod
            from trn_agent_boot.trn_boot import _ntff_profile_via_ctypes

            hook = _ntff_profile_via_ctypes("/opt/axon/libaxon_pjrt.so")
            mod.set_axon_ntff_profile_hook(hook)
    except Exception:
        pass


def run(x, w_project, w_out, trace=False):
    """Run the sharded kernel on hardware; returns (y [B,S,D] f32, results)."""
    x = np.asarray(x, dtype=np.float32)
    w_project = np.asarray(w_project, dtype=np.float32)
    w_out = np.asarray(w_out, dtype=np.float32)
    B, S, D = x.shape
    H = w_project.shape[0] // 3 // HD  # total heads
    HL = H // 2  # heads per core (2 cores per batch)
    n_cores = 2 * B
    DT, SB, ST = D // 128, S // SQB, S // 128

    nc = _get_program(S, D, HL, n_cores)

    in_maps = []
    for b in range(B):
        xt1, xt2 = _pretile_x(x[b], DT, SB, ST)
        for g in range(2):
            wqk, wv, wo = _pretile_weights(w_project, w_out, D, HL, g)
            in_maps.append({"xt1": xt1, "xt2": xt2, "wqk": wqk, "wv": wv, "wo": wo})

    if trace:
        _install_ntff_hook()
    res = bass_utils.run_bass_kernel_spmd(
        nc, in_maps, core_ids=list(range(n_cores)), trace=trace
    )
    # reassemble: ReduceScatter chunk g gives the even core rows
    # [g*CR, g*CR + CR/2) and the odd core the remaining half.
    CR = 4 * 128
    HG = CR // 2
    G = S // CR
    y = np.empty((B, S, D), np.float32)
    for b in range(B):
        y0 = res.results[2 * b]["y"]
        y1 = res.results[2 * b + 1]["y"]
        for g in range(G):
            y[b, g * CR : g * CR + HG] = y0[g * HG : (g + 1) * HG]
            y[b, g * CR + HG : (g + 1) * CR] = y1[g * HG : (g + 1) * HG]
    return y, res


def kernel(x, w_project, w_out):
    y, _ = run(x, w_project, w_out, trace=False)
    return y
